# revision 3
# baseline (speedup 1.0000x reference)
"""CrossConv2d (concat -> 3x3 conv -> BN -> +skip -> ReLU) on 8 Trainium2 cores.

Data-parallel over the fused (b*s)=32 batch axis: 4 images per core, all
four sharing one u (same b). The concat conv splits by input half:
out(b,s) = conv_u(u[b]) + conv_v(v[b,s]); conv_u is computed ONCE per core
and cached in SBUF as Z = conv_u(u) + BN shift (u-skip identity folded into
the center-tap weights), so per image only the K=64 v-half conv runs.

The tensor engine runs in 64x128 row-tiled mode: two independent K=64
matmuls execute concurrently (tile T0 = SBUF partitions 0-63, T8 = 64-127),
each accumulating into its own PSUM bank. Streams are paired so both tiles
stay busy: v0/v1 fixed on T0, v2/v3 on T8, and the once-per-core u stream
alternates tiles by chunk parity; emission strictly alternates T0/T8 so the
in-order tensor queue overlaps pairs (and hides LDWEIGHTS behind the
opposite tile). Per-core stream cost: 2.5 image-equivalents x 9 taps
instead of 4 x 9.

Everything is bf16 (inputs, weights, outputs; PSUM accumulates fp32),
halving DMA traffic; BN scale is folded into the weights, BN shift into Z,
and both skip identities into the center tap, so the per-image epilogue is
one VectorE add (psum += Z) and one ScalarE ReLU copy to bf16, like the
plain-conv version.
"""

import numpy as np
import ml_dtypes

import concourse.bacc as bacc
import concourse.mybir as mybir
from concourse import tile
from concourse.bass_utils import run_bass_kernel_spmd

EPS = 1e-5

B, S, C1, C2, H, W = 4, 8, 64, 64, 128, 128
CC = C1 + C2
N_CORES = 8
IMG_PER_CORE = (B * S) // N_CORES  # 4
WP = W + 2
HP = H + 2
NQ = H * WP                 # 16640 output cols (incl. junk pad cols)
NX = HP * WP + 4            # leading 1 + trailing 3 pad cells
TAPS = 9

F32 = mybir.dt.float32
BF16 = mybir.dt.bfloat16

_CACHE = {}

def _chunks():
    # 30 x 512 + 4 x 320 = 16640: EVEN chunk count so the T0/T8 pairing
    # stays balanced through the whole image
    out = [(512 * k, 512 * (k + 1)) for k in range(30)]
    q = 30 * 512
    while q < NQ:
        out.append((q, q + 320))
        q += 320
    return out


def _build_program():
    nc = bacc.Bacc(
        "TRN2", target_bir_lowering=False, debug=False, num_devices=N_CORES
    )
    u_d = nc.dram_tensor("u", [C1, HP * WP], BF16, kind="ExternalInput")
    v_d = nc.dram_tensor("v", [IMG_PER_CORE, C2, HP * WP], BF16, kind="ExternalInput")
    wu_d = nc.dram_tensor("wu", [CC, TAPS * CC], BF16, kind="ExternalInput")
    wv_d = nc.dram_tensor("wv", [CC, TAPS * CC], BF16, kind="ExternalInput")
    sh_d = nc.dram_tensor("shift", [CC, 1], F32, kind="ExternalInput")
    o_d = nc.dram_tensor("o", [IMG_PER_CORE, CC, NQ], BF16, kind="ExternalOutput")

    chunks = _chunks()

    with tile.TileContext(nc) as tc:
        with (
            tc.tile_pool(name="consts", bufs=1) as cpool,
            tc.tile_pool(name="ostrip", bufs=8) as opool,
            tc.tile_pool(name="psum", bufs=8, space="PSUM") as ppool,
        ):
            # whole padded images, resident for the whole kernel:
            #   xu: u on BOTH partition halves (T0 and T8 copies)
            #   xa: v0 on partitions 0-63, v2 on 64-127
            #   xb: v1 on partitions 0-63, v3 on 64-127
            xu = cpool.tile([CC, NX], BF16)
            xa = cpool.tile([CC, NX], BF16)
            xb = cpool.tile([CC, NX], BF16)
            zz = cpool.tile([CC, NQ], BF16)
            wu = cpool.tile([CC, TAPS * CC], BF16)
            wv = cpool.tile([CC, TAPS * CC], BF16)
            sh = cpool.tile([CC, 1], F32)

            # startup critical path: tap-0 weights + the leading pad cells
            # + block 0 of u-lo / v2 gate the first matmul pair
            nc.scalar.dma_start(wu[:, 0:CC], wu_d[:, 0:CC])
            nc.scalar.dma_start(wv[:, 0:CC], wv_d[:, 0:CC])
            for t_ in (xu, xa, xb):
                nc.scalar.dma_start(t_[0:C1, 0:4], u_d[:, 0:4])
                nc.scalar.dma_start(t_[C1:CC, 0:4], u_d[:, 0:4])
            for t in range(1, TAPS):
                nc.scalar.dma_start(wu[:, t * CC:(t + 1) * CC], wu_d[:, t * CC:(t + 1) * CC])
                nc.scalar.dma_start(wv[:, t * CC:(t + 1) * CC], wv_d[:, t * CC:(t + 1) * CC])
            nc.scalar.dma_start(sh[:], sh_d[:])
            for t_ in (xu, xa, xb):
                nc.scalar.dma_start(t_[0:C1, 1 + HP * WP:], u_d[:, 0:3])
                nc.scalar.dma_start(t_[C1:CC, 1 + HP * WP:], u_d[:, 0:3])

            NBLK = 16
            blk = [(HP * WP * k // NBLK, HP * WP * (k + 1) // NBLK)
                   for k in range(NBLK)]
            for b0, b1 in blk:
                # consumption order: u-lo (chunk-even u), v2/v0 (first zip
                # pairs), then the rest
                nc.sync.dma_start(xu[0:C1, 1 + b0:1 + b1], u_d[:, b0:b1])
                nc.sync.dma_start(xa[C1:CC, 1 + b0:1 + b1], v_d[2, :, b0:b1])
                nc.sync.dma_start(xa[0:C1, 1 + b0:1 + b1], v_d[0, :, b0:b1])
                nc.sync.dma_start(xb[C1:CC, 1 + b0:1 + b1], v_d[3, :, b0:b1])
                nc.sync.dma_start(xb[0:C1, 1 + b0:1 + b1], v_d[1, :, b0:b1])
                nc.sync.dma_start(xu[C1:CC, 1 + b0:1 + b1], u_d[:, b0:b1])

            # image -> (tile half, source tile): v0,v1 on T0; v2,v3 on T8
            vhalf = {0: (0, xa), 1: (0, xb), 2: (1, xa), 3: (1, xb)}

            def mm_thunk(ps, wtile, half, xtile, t, q0, q1, start, stop):
                p0 = half * C1
                dy, dx = t // 3 - 1, t % 3 - 1
                off = 1 + (1 + dy) * WP + dx

                def emit():
                    nc.tensor.matmul(
                        ps[:, 0:q1 - q0],
                        wtile[p0:p0 + C1, t * CC:(t + 1) * CC],
                        xtile[p0:p0 + C1, q0 + off:q1 + off],
                        start=start, stop=stop,
                    )
                return emit

            # process chunks in parity pairs; strict T0/T8 alternation
            for ce in range(0, len(chunks), 2):
                pair = [ce] if ce + 1 >= len(chunks) else [ce, ce + 1]
                lists = {0: [], 1: []}
                drains = []
                for ci in pair:
                    q0, q1 = chunks[ci]
                    n = q1 - q0
                    up = ci % 2  # u's tile half this chunk
                    ps_u = ppool.tile([CC, 512], F32, tag="ps", name="ps_u")
                    psv = []
                    for img in range(IMG_PER_CORE):
                        psv.append(ppool.tile([CC, 512], F32, tag="ps", name="ps_v"))
                    for t in range(TAPS):
                        lists[up].append(mm_thunk(
                            ps_u, wu, up, xu, t, q0, q1, t == 0, t == TAPS - 1))
                    for img in range(IMG_PER_CORE):
                        hf, xt = vhalf[img]
                        for t in range(TAPS):
                            lists[hf].append(mm_thunk(
                                psv[img], wv, hf, xt, t, q0, q1,
                                t == 0, t == TAPS - 1))

                    def drain(ci=ci, q0=q0, q1=q1, n=n, ps_u=ps_u, psv=psv):
                        nc.scalar.add(zz[:, q0:q1], ps_u[:, 0:n], sh[:])
                        last_img_chunk = ci >= len(chunks) - 2
                        for img in range(IMG_PER_CORE):
                            og = opool.tile([CC, 512], BF16, tag="og")
                            nc.vector.tensor_add(
                                psv[img][:, 0:n], psv[img][:, 0:n], zz[:, q0:q1])
                            nc.scalar.activation(
                                og[:, 0:n], psv[img][:, 0:n],
                                mybir.ActivationFunctionType.Relu)
                            oeng = nc.sync if last_img_chunk else nc.gpsimd
                            oeng.dma_start(o_d[img, :, q0:q1], og[:, 0:n])
                    drains.append(drain)

                # strict alternation keeps both tiles streaming and lets
                # LDWEIGHTS hide behind the opposite tile's matmul
                l0, l8 = lists[0], lists[1]
                for i in range(max(len(l0), len(l8))):
                    if i < len(l0):
                        l0[i]()
                    if i < len(l8):
                        l8[i]()
                for drain in drains:
                    drain()
    nc.compile()
    return nc


def _get_program():
    if "nc" not in _CACHE:
        _CACHE["nc"] = _build_program()
    return _CACHE["nc"]


def _prep_inputs(u, v, conv_w, bn_gamma, bn_beta, bn_mean, bn_var):
    u = np.asarray(u, dtype=np.float32)
    v = np.asarray(v, dtype=np.float32)
    conv_w = np.asarray(conv_w, dtype=np.float32)
    bn_gamma = np.asarray(bn_gamma, dtype=np.float32)
    bn_beta = np.asarray(bn_beta, dtype=np.float32)
    bn_mean = np.asarray(bn_mean, dtype=np.float32)
    bn_var = np.asarray(bn_var, dtype=np.float32)

    scale = bn_gamma / np.sqrt(bn_var + EPS)
    shift = (bn_beta - bn_mean * scale).astype(np.float32).reshape(CC, 1)
    wsc = (conv_w * scale[:, None, None, None]).astype(np.float32)
    # skip = identity on the center tap (ky=kx=1), NOT BN-scaled
    wsk = wsc.copy()
    wsk[:, :, 1, 1] += np.eye(CC, dtype=np.float32)
    # lhsT layout per tap t = ky*3+kx: w[i, t*CC + o] = wsk[o, i, ky, kx]
    w_lhsT = np.ascontiguousarray(wsk.transpose(1, 2, 3, 0).reshape(CC, TAPS * CC))
    wu_host = np.concatenate([w_lhsT[0:C1], w_lhsT[0:C1]], axis=0)
    wv_host = np.concatenate([w_lhsT[C1:CC], w_lhsT[C1:CC]], axis=0)
    wu_host = wu_host.astype(ml_dtypes.bfloat16)
    wv_host = wv_host.astype(ml_dtypes.bfloat16)

    in_maps = []
    for m in range(N_CORES):
        b = m // 2
        s0 = (m % 2) * IMG_PER_CORE
        u_pad = np.zeros((C1, HP, WP), np.float32)
        u_pad[:, 1:1 + H, 1:1 + W] = u[b, 0]
        v_pad = np.zeros((IMG_PER_CORE, C2, HP, WP), np.float32)
        v_pad[:, :, 1:1 + H, 1:1 + W] = v[b, s0:s0 + IMG_PER_CORE]
        in_maps.append(
            {
                "u": u_pad.reshape(C1, HP * WP).astype(ml_dtypes.bfloat16),
                "v": v_pad.reshape(IMG_PER_CORE, C2, HP * WP).astype(ml_dtypes.bfloat16),
                "wu": wu_host,
                "wv": wv_host,
                "shift": shift,
            }
        )
    return in_maps


def _run(inputs, trace=False):
    nc = _get_program()
    in_maps = _prep_inputs(**inputs)
    res = run_bass_kernel_spmd(
        nc, in_maps, list(range(N_CORES)), trace=trace
    )
    out = np.empty((B, 1, S, CC, H, W), np.float32)
    for m in range(N_CORES):
        b = m // 2
        s0 = (m % 2) * IMG_PER_CORE
        o_pad = np.asarray(res.results[m]["o"], dtype=np.float32).reshape(
            IMG_PER_CORE, CC, H, WP)
        out[b, 0, s0:s0 + IMG_PER_CORE] = o_pad[:, :, :, 1:1 + W]
    return out, res


def kernel(u, v, conv_w, bn_gamma, bn_beta, bn_mean, bn_var):
    out, _ = _run(
        dict(
            u=u,
            v=v,
            conv_w=conv_w,
            bn_gamma=bn_gamma,
            bn_beta=bn_beta,
            bn_mean=bn_mean,
            bn_var=bn_var,
        )
    )
    return out


# revision 6
# speedup vs baseline: 1.0383x; 1.0383x over previous
"""CrossConv2d (concat -> 3x3 conv -> BN -> +skip -> ReLU) on 8 Trainium2 cores.

Data-parallel over the fused (b*s)=32 batch axis: 4 images per core, all
four sharing one u (same b). The concat conv splits by input half:
out(b,s) = conv_u(u[b]) + conv_v(v[b,s]); conv_u is computed ONCE per core
and cached in SBUF as Z = conv_u(u) + BN shift (u-skip identity folded into
the center-tap weights), so per image only the K=64 v-half conv runs.

The tensor engine runs in 64x128 row-tiled mode: two independent K=64
matmuls execute concurrently (tile T0 = SBUF partitions 0-63, T8 = 64-127),
each accumulating into its own PSUM bank. Streams are paired so both tiles
stay busy: v0/v1 fixed on T0, v2/v3 on T8, and the once-per-core u stream
alternates tiles by chunk parity; emission strictly alternates T0/T8 so the
in-order tensor queue overlaps pairs (and hides LDWEIGHTS behind the
opposite tile). Per-core stream cost: 2.5 image-equivalents x 9 taps
instead of 4 x 9.

Everything is bf16 (inputs, weights, outputs; PSUM accumulates fp32),
halving DMA traffic; BN scale is folded into the weights, BN shift into Z,
and both skip identities into the center tap, so the per-image epilogue is
one VectorE add (psum += Z) and one ScalarE ReLU copy to bf16, like the
plain-conv version.
"""

import numpy as np
import ml_dtypes

import concourse.bacc as bacc
import concourse.mybir as mybir
from concourse import tile
from concourse.bass_utils import run_bass_kernel_spmd

EPS = 1e-5

B, S, C1, C2, H, W = 4, 8, 64, 64, 128, 128
CC = C1 + C2
N_CORES = 8
IMG_PER_CORE = (B * S) // N_CORES  # 4
WP = W + 2
HP = H + 2
NQ = H * WP                 # 16640 output cols (incl. junk pad cols)
NX = HP * WP + 4            # leading 1 + trailing 3 pad cells
TAPS = 9

F32 = mybir.dt.float32
BF16 = mybir.dt.bfloat16

_CACHE = {}

def _chunks():
    # 30 x 512 + 4 x 320 = 16640: EVEN chunk count so the T0/T8 pairing
    # stays balanced through the whole image
    out = [(512 * k, 512 * (k + 1)) for k in range(30)]
    q = 30 * 512
    while q < NQ:
        out.append((q, q + 320))
        q += 320
    return out


def _build_program():
    nc = bacc.Bacc(
        "TRN2", target_bir_lowering=False, debug=False, num_devices=N_CORES
    )
    u_d = nc.dram_tensor("u", [C1, HP * WP], BF16, kind="ExternalInput")
    v_d = nc.dram_tensor("v", [IMG_PER_CORE, C2, HP * WP], BF16, kind="ExternalInput")
    wu_d = nc.dram_tensor("wu", [CC, TAPS * CC], BF16, kind="ExternalInput")
    wv_d = nc.dram_tensor("wv", [CC, TAPS * CC], BF16, kind="ExternalInput")
    sh_d = nc.dram_tensor("shift", [CC, 1], F32, kind="ExternalInput")
    o_d = nc.dram_tensor("o", [IMG_PER_CORE, CC, NQ], BF16, kind="ExternalOutput")

    chunks = _chunks()

    with tile.TileContext(nc) as tc:
        with (
            tc.tile_pool(name="consts", bufs=1) as cpool,
            tc.tile_pool(name="ostrip", bufs=8) as opool,
            tc.tile_pool(name="psum", bufs=8, space="PSUM") as ppool,
        ):
            # whole padded images, resident for the whole kernel:
            #   xu: u on BOTH partition halves (T0 and T8 copies)
            #   xa: v0 on partitions 0-63, v2 on 64-127
            #   xb: v1 on partitions 0-63, v3 on 64-127
            xu = cpool.tile([CC, NX], BF16)
            xa = cpool.tile([CC, NX], BF16)
            xb = cpool.tile([CC, NX], BF16)
            zz = cpool.tile([CC, NQ], BF16)
            wu = cpool.tile([CC, TAPS * CC], BF16)
            wv = cpool.tile([CC, TAPS * CC], BF16)
            sh = cpool.tile([CC, 1], F32)

            # startup critical path, in gating order of the zip stream:
            # pos 0 = u(c0)@T0 (xu-lo pad + wu t0 + u-lo b0) paired with
            # v2(c0)@T8 (xa-hi pad + wv t0 + v2 b0). Triggers are spread
            # over four queues so the first ~3 blocks land in parallel:
            #   scalar: leading pads + wu taps; vector: wv taps;
            #   sync: T0-half blocks (+bulk); gpsimd: T8-half early blocks.
            nc.scalar.dma_start(xu[0:C1, 0:4], u_d[:, 0:4])
            nc.scalar.dma_start(xa[C1:CC, 0:4], u_d[:, 0:4])
            nc.scalar.dma_start(wu[:], wu_d[:])
            nc.gpsimd.dma_start(wv[:], wv_d[:])
            nc.scalar.dma_start(xa[0:C1, 0:4], u_d[:, 0:4])
            nc.scalar.dma_start(xb[C1:CC, 0:4], u_d[:, 0:4])
            nc.scalar.dma_start(xb[0:C1, 0:4], u_d[:, 0:4])
            nc.scalar.dma_start(xu[C1:CC, 0:4], u_d[:, 0:4])
            nc.scalar.dma_start(sh[:], sh_d[:])
            for t_ in (xu, xa, xb):
                nc.scalar.dma_start(t_[0:C1, 1 + HP * WP:], u_d[:, 0:3])
                nc.scalar.dma_start(t_[C1:CC, 1 + HP * WP:], u_d[:, 0:3])

            NBLK = 16
            blk = [(HP * WP * k // NBLK, HP * WP * (k + 1) // NBLK)
                   for k in range(NBLK)]
            for k, (b0, b1) in enumerate(blk):
                # T0-half sources on sync, T8-half on gpsimd for the first
                # blocks (gpsimd is free until output stores begin ~16us);
                # the bulk goes to sync whose input backlog drains by ~60us
                heng = nc.gpsimd if k < 3 else nc.sync
                nc.sync.dma_start(xu[0:C1, 1 + b0:1 + b1], u_d[:, b0:b1])
                heng.dma_start(xa[C1:CC, 1 + b0:1 + b1], v_d[2, :, b0:b1])
                nc.sync.dma_start(xa[0:C1, 1 + b0:1 + b1], v_d[0, :, b0:b1])
                heng.dma_start(xb[C1:CC, 1 + b0:1 + b1], v_d[3, :, b0:b1])
                nc.sync.dma_start(xb[0:C1, 1 + b0:1 + b1], v_d[1, :, b0:b1])
                heng.dma_start(xu[C1:CC, 1 + b0:1 + b1], u_d[:, b0:b1])

            # image -> (tile half, source tile): v0,v1 on T0; v2,v3 on T8
            vhalf = {0: (0, xa), 1: (0, xb), 2: (1, xa), 3: (1, xb)}

            def mm_thunk(ps, wtile, half, xtile, t, q0, q1, start, stop):
                p0 = half * C1
                dy, dx = t // 3 - 1, t % 3 - 1
                off = 1 + (1 + dy) * WP + dx

                def emit():
                    nc.tensor.matmul(
                        ps[:, 0:q1 - q0],
                        wtile[p0:p0 + C1, t * CC:(t + 1) * CC],
                        xtile[p0:p0 + C1, q0 + off:q1 + off],
                        start=start, stop=stop,
                    )
                return emit

            # process chunks in parity pairs; strict T0/T8 alternation
            for ce in range(0, len(chunks), 2):
                pair = [ce] if ce + 1 >= len(chunks) else [ce, ce + 1]
                lists = {0: [], 1: []}
                drains = []
                for ci in pair:
                    q0, q1 = chunks[ci]
                    n = q1 - q0
                    up = ci % 2  # u's tile half this chunk
                    ps_u = ppool.tile([CC, 512], F32, tag="ps", name="ps_u")
                    psv = []
                    for img in range(IMG_PER_CORE):
                        psv.append(ppool.tile([CC, 512], F32, tag="ps", name="ps_v"))
                    for t in range(TAPS):
                        lists[up].append(mm_thunk(
                            ps_u, wu, up, xu, t, q0, q1, t == 0, t == TAPS - 1))
                    for img in range(IMG_PER_CORE):
                        hf, xt = vhalf[img]
                        for t in range(TAPS):
                            lists[hf].append(mm_thunk(
                                psv[img], wv, hf, xt, t, q0, q1,
                                t == 0, t == TAPS - 1))

                    def drain(ci=ci, q0=q0, q1=q1, n=n, ps_u=ps_u, psv=psv):
                        nc.scalar.add(zz[:, q0:q1], ps_u[:, 0:n], sh[:])
                        # final chunk's stores alternate sync/scalar so the
                        # trailing triggers don't serialize on one queue;
                        # everything else rides gpsimd
                        if ci == len(chunks) - 1:
                            oengs = [nc.sync, nc.scalar, nc.sync, nc.scalar]
                        elif ci == len(chunks) - 2:
                            oengs = [nc.sync] * 4
                        else:
                            oengs = [nc.gpsimd] * 4
                        for img in range(IMG_PER_CORE):
                            og = opool.tile([CC, 512], BF16, tag="og")
                            nc.vector.tensor_add(
                                psv[img][:, 0:n], psv[img][:, 0:n], zz[:, q0:q1])
                            nc.scalar.activation(
                                og[:, 0:n], psv[img][:, 0:n],
                                mybir.ActivationFunctionType.Relu)
                            oengs[img].dma_start(o_d[img, :, q0:q1], og[:, 0:n])
                    drains.append(drain)

                # strict alternation keeps both tiles streaming and lets
                # LDWEIGHTS hide behind the opposite tile's matmul
                l0, l8 = lists[0], lists[1]
                for i in range(max(len(l0), len(l8))):
                    if i < len(l0):
                        l0[i]()
                    if i < len(l8):
                        l8[i]()
                for drain in drains:
                    drain()
    nc.compile()
    return nc


def _get_program():
    if "nc" not in _CACHE:
        _CACHE["nc"] = _build_program()
    return _CACHE["nc"]


def _prep_inputs(u, v, conv_w, bn_gamma, bn_beta, bn_mean, bn_var):
    u = np.asarray(u, dtype=np.float32)
    v = np.asarray(v, dtype=np.float32)
    conv_w = np.asarray(conv_w, dtype=np.float32)
    bn_gamma = np.asarray(bn_gamma, dtype=np.float32)
    bn_beta = np.asarray(bn_beta, dtype=np.float32)
    bn_mean = np.asarray(bn_mean, dtype=np.float32)
    bn_var = np.asarray(bn_var, dtype=np.float32)

    scale = bn_gamma / np.sqrt(bn_var + EPS)
    shift = (bn_beta - bn_mean * scale).astype(np.float32).reshape(CC, 1)
    wsc = (conv_w * scale[:, None, None, None]).astype(np.float32)
    # skip = identity on the center tap (ky=kx=1), NOT BN-scaled
    wsk = wsc.copy()
    wsk[:, :, 1, 1] += np.eye(CC, dtype=np.float32)
    # lhsT layout per tap t = ky*3+kx: w[i, t*CC + o] = wsk[o, i, ky, kx]
    w_lhsT = np.ascontiguousarray(wsk.transpose(1, 2, 3, 0).reshape(CC, TAPS * CC))
    wu_host = np.concatenate([w_lhsT[0:C1], w_lhsT[0:C1]], axis=0)
    wv_host = np.concatenate([w_lhsT[C1:CC], w_lhsT[C1:CC]], axis=0)
    wu_host = wu_host.astype(ml_dtypes.bfloat16)
    wv_host = wv_host.astype(ml_dtypes.bfloat16)

    in_maps = []
    for m in range(N_CORES):
        b = m // 2
        s0 = (m % 2) * IMG_PER_CORE
        u_pad = np.zeros((C1, HP, WP), np.float32)
        u_pad[:, 1:1 + H, 1:1 + W] = u[b, 0]
        v_pad = np.zeros((IMG_PER_CORE, C2, HP, WP), np.float32)
        v_pad[:, :, 1:1 + H, 1:1 + W] = v[b, s0:s0 + IMG_PER_CORE]
        in_maps.append(
            {
                "u": u_pad.reshape(C1, HP * WP).astype(ml_dtypes.bfloat16),
                "v": v_pad.reshape(IMG_PER_CORE, C2, HP * WP).astype(ml_dtypes.bfloat16),
                "wu": wu_host,
                "wv": wv_host,
                "shift": shift,
            }
        )
    return in_maps


def _run(inputs, trace=False):
    nc = _get_program()
    in_maps = _prep_inputs(**inputs)
    res = run_bass_kernel_spmd(
        nc, in_maps, list(range(N_CORES)), trace=trace
    )
    out = np.empty((B, 1, S, CC, H, W), np.float32)
    for m in range(N_CORES):
        b = m // 2
        s0 = (m % 2) * IMG_PER_CORE
        o_pad = np.asarray(res.results[m]["o"], dtype=np.float32).reshape(
            IMG_PER_CORE, CC, H, WP)
        out[b, 0, s0:s0 + IMG_PER_CORE] = o_pad[:, :, :, 1:1 + W]
    return out, res


def kernel(u, v, conv_w, bn_gamma, bn_beta, bn_mean, bn_var):
    out, _ = _run(
        dict(
            u=u,
            v=v,
            conv_w=conv_w,
            bn_gamma=bn_gamma,
            bn_beta=bn_beta,
            bn_mean=bn_mean,
            bn_var=bn_var,
        )
    )
    return out


# revision 7
# speedup vs baseline: 1.0704x; 1.0308x over previous
"""CrossConv2d (concat -> 3x3 conv -> BN -> +skip -> ReLU) on 8 Trainium2 cores.

Data-parallel over the fused (b*s)=32 batch axis: 4 images per core, all
four sharing one u (same b). The concat conv splits by input half:
out(b,s) = conv_u(u[b]) + conv_v(v[b,s]); conv_u is computed ONCE per core
and cached in SBUF as Z = conv_u(u) + BN shift (u-skip identity folded into
the center-tap weights), so per image only the K=64 v-half conv runs.

The tensor engine runs in 64x128 row-tiled mode: two independent K=64
matmuls execute concurrently (tile T0 = SBUF partitions 0-63, T8 = 64-127),
each accumulating into its own PSUM bank. Streams are paired so both tiles
stay busy: v0/v1 fixed on T0, v2/v3 on T8, and the once-per-core u stream
alternates tiles by chunk parity; emission strictly alternates T0/T8 so the
in-order tensor queue overlaps pairs (and hides LDWEIGHTS behind the
opposite tile). Per-core stream cost: 2.5 image-equivalents x 9 taps
instead of 4 x 9.

Outputs are PACKED: each 512-col chunk is 4 image rows x 128 cols via 3-D
rhs access patterns into the padded [HP, WP] input planes, so no junk
columns are computed, stored, or sliced, and no leading/trailing pad cells
are needed in SBUF.

Everything is bf16 (inputs, weights, outputs; PSUM accumulates fp32),
halving DMA traffic; BN scale is folded into the weights, BN shift into Z,
and both skip identities into the center tap, so the per-image epilogue is
one VectorE add (psum += Z) and one ScalarE ReLU copy to bf16.
"""

import numpy as np
import ml_dtypes

import concourse.bacc as bacc
import concourse.mybir as mybir
from concourse import tile
from concourse.bass_utils import run_bass_kernel_spmd

EPS = 1e-5

B, S, C1, C2, H, W = 4, 8, 64, 64, 128, 128
CC = C1 + C2
N_CORES = 8
IMG_PER_CORE = (B * S) // N_CORES  # 4
WP = W + 2
HP = H + 2
NQ = H * W                  # 16384 packed output cols
TAPS = 9
ROWS_PER_CHUNK = 4          # 4 x 128 = 512 = one PSUM bank
NCHUNK = H // ROWS_PER_CHUNK  # 32 (even: keeps T0/T8 pairing balanced)

F32 = mybir.dt.float32
BF16 = mybir.dt.bfloat16

_CACHE = {}


def _build_program():
    nc = bacc.Bacc(
        "TRN2", target_bir_lowering=False, debug=False, num_devices=N_CORES
    )
    u_d = nc.dram_tensor("u", [C1, HP, WP], BF16, kind="ExternalInput")
    v_d = nc.dram_tensor("v", [IMG_PER_CORE, C2, HP, WP], BF16, kind="ExternalInput")
    wu_d = nc.dram_tensor("wu", [CC, TAPS * CC], BF16, kind="ExternalInput")
    wv_d = nc.dram_tensor("wv", [CC, TAPS * CC], BF16, kind="ExternalInput")
    sh_d = nc.dram_tensor("shift", [CC, 1], F32, kind="ExternalInput")
    o_d = nc.dram_tensor("o", [IMG_PER_CORE, CC, NQ], BF16, kind="ExternalOutput")

    with tile.TileContext(nc) as tc:
        with (
            tc.tile_pool(name="consts", bufs=1) as cpool,
            tc.tile_pool(name="ostrip", bufs=8) as opool,
            tc.tile_pool(name="psum", bufs=8, space="PSUM") as ppool,
        ):
            # whole padded images, resident for the whole kernel:
            #   xu: u on BOTH partition halves (T0 and T8 copies)
            #   xa: v0 on partitions 0-63, v2 on 64-127
            #   xb: v1 on partitions 0-63, v3 on 64-127
            xu = cpool.tile([CC, HP, WP], BF16)
            xa = cpool.tile([CC, HP, WP], BF16)
            xb = cpool.tile([CC, HP, WP], BF16)
            zz = cpool.tile([CC, NQ], BF16)
            wu = cpool.tile([CC, TAPS * CC], BF16)
            wv = cpool.tile([CC, TAPS * CC], BF16)
            sh = cpool.tile([CC, 1], F32)

            # weight loads split 32 partition-rows apiece: a 2D DMA's rows
            # run serially on one engine (~45ns/row), separate DMAs run on
            # separate engines. First the halves that gate zip position 0
            # (wu rows 0:64 for u(c0)@T0, wv rows 64:128 for v2(c0)@T8).
            for p in (0, 32, 64, 96):
                nc.scalar.dma_start(wu[p:p + 32, :], wu_d[p:p + 32, :])
            for p in (64, 96, 0, 32):
                nc.gpsimd.dma_start(wv[p:p + 32, :], wv_d[p:p + 32, :])
            nc.scalar.dma_start(sh[:], sh_d[:])

            # input planes, 13 row-blocks of 10 padded rows; T0-half
            # sources on sync, T8-half early blocks on gpsimd (it is free
            # until output stores begin), T8 bulk also on sync
            NBLK = 13
            for k in range(NBLK):
                r0, r1 = 10 * k, min(10 * k + 10, HP)
                heng = nc.gpsimd if k < 3 else nc.sync
                nc.sync.dma_start(xu[0:C1, r0:r1, :], u_d[:, r0:r1, :])
                heng.dma_start(xa[C1:CC, r0:r1, :], v_d[2, :, r0:r1, :])
                nc.sync.dma_start(xa[0:C1, r0:r1, :], v_d[0, :, r0:r1, :])
                heng.dma_start(xb[C1:CC, r0:r1, :], v_d[3, :, r0:r1, :])
                nc.sync.dma_start(xb[0:C1, r0:r1, :], v_d[1, :, r0:r1, :])
                heng.dma_start(xu[C1:CC, r0:r1, :], u_d[:, r0:r1, :])

            # image -> (tile half, source tile): v0,v1 on T0; v2,v3 on T8
            vhalf = {0: (0, xa), 1: (0, xb), 2: (1, xa), 3: (1, xb)}

            def mm_thunk(ps, wtile, half, xtile, t, ci, start, stop):
                p0 = half * C1
                dy, dx = t // 3 - 1, t % 3 - 1
                ra = ROWS_PER_CHUNK * ci + 1 + dy

                def emit():
                    nc.tensor.matmul(
                        ps[:],
                        wtile[p0:p0 + C1, t * CC:(t + 1) * CC],
                        xtile[p0:p0 + C1, ra:ra + ROWS_PER_CHUNK, 1 + dx:1 + dx + W],
                        start=start, stop=stop,
                    )
                return emit

            # process chunks in parity pairs; strict T0/T8 alternation
            for ce in range(0, NCHUNK, 2):
                lists = {0: [], 1: []}
                drains = []
                for ci in (ce, ce + 1):
                    q0, q1 = 512 * ci, 512 * ci + 512
                    up = ci % 2  # u's tile half this chunk
                    ps_u = ppool.tile([CC, 512], F32, tag="ps", name="ps_u")
                    psv = []
                    for img in range(IMG_PER_CORE):
                        psv.append(ppool.tile([CC, 512], F32, tag="ps", name="ps_v"))
                    for t in range(TAPS):
                        lists[up].append(mm_thunk(
                            ps_u, wu, up, xu, t, ci, t == 0, t == TAPS - 1))
                    for img in range(IMG_PER_CORE):
                        hf, xt = vhalf[img]
                        for t in range(TAPS):
                            lists[hf].append(mm_thunk(
                                psv[img], wv, hf, xt, t, ci,
                                t == 0, t == TAPS - 1))

                    def drain(ci=ci, q0=q0, q1=q1, ps_u=ps_u, psv=psv):
                        nc.scalar.add(zz[:, q0:q1], ps_u[:], sh[:])
                        # drain images in the order their matmuls stop
                        # (earlier PSUM release, shorter exposed tail);
                        # final pair's stores ride the then-idle sync queue
                        order = [2, 0, 3, 1] if ci % 2 == 0 else [0, 2, 1, 3]
                        last_pair = ci >= NCHUNK - 2
                        for img in order:
                            og = opool.tile([CC, 512], BF16, tag="og")
                            nc.vector.tensor_add(
                                psv[img][:], psv[img][:], zz[:, q0:q1])
                            nc.scalar.activation(
                                og[:], psv[img][:],
                                mybir.ActivationFunctionType.Relu)
                            oeng = nc.sync if last_pair else nc.gpsimd
                            oeng.dma_start(o_d[img, :, q0:q1], og[:])
                    drains.append(drain)

                # strict alternation keeps both tiles streaming and lets
                # LDWEIGHTS hide behind the opposite tile's matmul
                l0, l8 = lists[0], lists[1]
                for i in range(max(len(l0), len(l8))):
                    if i < len(l0):
                        l0[i]()
                    if i < len(l8):
                        l8[i]()
                for drain in drains:
                    drain()
    nc.compile()
    return nc


def _get_program():
    if "nc" not in _CACHE:
        _CACHE["nc"] = _build_program()
    return _CACHE["nc"]


def _prep_inputs(u, v, conv_w, bn_gamma, bn_beta, bn_mean, bn_var):
    u = np.asarray(u, dtype=np.float32)
    v = np.asarray(v, dtype=np.float32)
    conv_w = np.asarray(conv_w, dtype=np.float32)
    bn_gamma = np.asarray(bn_gamma, dtype=np.float32)
    bn_beta = np.asarray(bn_beta, dtype=np.float32)
    bn_mean = np.asarray(bn_mean, dtype=np.float32)
    bn_var = np.asarray(bn_var, dtype=np.float32)

    scale = bn_gamma / np.sqrt(bn_var + EPS)
    shift = (bn_beta - bn_mean * scale).astype(np.float32).reshape(CC, 1)
    wsc = (conv_w * scale[:, None, None, None]).astype(np.float32)
    # skip = identity on the center tap (ky=kx=1), NOT BN-scaled
    wsk = wsc.copy()
    wsk[:, :, 1, 1] += np.eye(CC, dtype=np.float32)
    # lhsT layout per tap t = ky*3+kx: w[i, t*CC + o] = wsk[o, i, ky, kx]
    w_lhsT = np.ascontiguousarray(wsk.transpose(1, 2, 3, 0).reshape(CC, TAPS * CC))
    wu_host = np.concatenate([w_lhsT[0:C1], w_lhsT[0:C1]], axis=0)
    wv_host = np.concatenate([w_lhsT[C1:CC], w_lhsT[C1:CC]], axis=0)
    wu_host = wu_host.astype(ml_dtypes.bfloat16)
    wv_host = wv_host.astype(ml_dtypes.bfloat16)

    in_maps = []
    for m in range(N_CORES):
        b = m // 2
        s0 = (m % 2) * IMG_PER_CORE
        u_pad = np.zeros((C1, HP, WP), np.float32)
        u_pad[:, 1:1 + H, 1:1 + W] = u[b, 0]
        v_pad = np.zeros((IMG_PER_CORE, C2, HP, WP), np.float32)
        v_pad[:, :, 1:1 + H, 1:1 + W] = v[b, s0:s0 + IMG_PER_CORE]
        in_maps.append(
            {
                "u": u_pad.astype(ml_dtypes.bfloat16),
                "v": v_pad.astype(ml_dtypes.bfloat16),
                "wu": wu_host,
                "wv": wv_host,
                "shift": shift,
            }
        )
    return in_maps


def _run(inputs, trace=False):
    nc = _get_program()
    in_maps = _prep_inputs(**inputs)
    res = run_bass_kernel_spmd(
        nc, in_maps, list(range(N_CORES)), trace=trace
    )
    out = np.empty((B, 1, S, CC, H, W), np.float32)
    for m in range(N_CORES):
        b = m // 2
        s0 = (m % 2) * IMG_PER_CORE
        out[b, 0, s0:s0 + IMG_PER_CORE] = np.asarray(
            res.results[m]["o"], dtype=np.float32).reshape(IMG_PER_CORE, CC, H, W)
    return out, res


def kernel(u, v, conv_w, bn_gamma, bn_beta, bn_mean, bn_var):
    out, _ = _run(
        dict(
            u=u,
            v=v,
            conv_w=conv_w,
            bn_gamma=bn_gamma,
            bn_beta=bn_beta,
            bn_mean=bn_mean,
            bn_var=bn_var,
        )
    )
    return out
